# revision 1
# baseline (speedup 1.0000x reference)
"""Trainium2 Bass kernel for nn_DendriticLayerSiLU_Template.

out = silu(g) * (x @ W.T), where per (token n, unit h):
  a[n,h,w] = sum_s x[n, w*64+s] * T[h, w*64+s]      (W=32 windows of size 64)
  p = softmax(|a| / tau), tau=1  (over w)
  g[n,h] = sum_w p[n,h,w] * a[n,h,w]

Strategy: 8-way data-parallel over N=4096 tokens (512/core). Host converts
x/W/T to fp16 and pre-transposes so every matmul operand has its contraction
dim on partitions. On-chip per core, per (token-tile, h-chunk) unit:
  - einsum: 32 window matmuls [K=64, M=128 tok, N=512 h] -> PSUM fp32,
    with the 16 linear k-chunk matmuls interleaved to keep PE smooth
  - gate:  ACT copies PSUM pairs to fp16, DVE abs (int16 mask, 4x),
    ACT exp (same table set as tanh -> no table switches), DVE mul (2x),
    in-place pairwise reduction trees over windows for num/den,
    fast-reciprocal, g = num/den
  - finish: out = g*lin*(1+tanh(g/2))/2  (= silu(g)*lin), DMA out
"""

import sys

if "/opt/trn_rl_repo" not in sys.path:
    sys.path.insert(0, "/opt/trn_rl_repo")

import numpy as np

import concourse.bass as bass
import concourse.tile as tile
from concourse import bacc, mybir
from concourse.bass_utils import run_bass_kernel_spmd

# Problem shapes (hardcoded per harness contract)
N_TOKENS = 4096
D = 2048          # in_features
H = 1024          # out_features
WIN = 64          # window size
NW = 32           # num windows
NCORES = 8
TOK = N_TOKENS // NCORES    # tokens per core = 512
NTT = TOK // 128            # token tiles per core = 4
NHC = H // 512              # h chunks = 2
HALF = 16                   # windows per half
KC = D // 128               # k chunks for linear = 16

F16 = mybir.dt.float16
F32 = mybir.dt.float32


def _build_module():
    nc = bacc.Bacc("TRN2", target_bir_lowering=False, debug=False,
                   num_devices=NCORES)

    xT = nc.dram_tensor("xT", [D, TOK], F16, kind="ExternalInput").ap()
    wT = nc.dram_tensor("wT", [D, H], F16, kind="ExternalInput").ap()
    tT = nc.dram_tensor("tT", [D, H], F16, kind="ExternalInput").ap()
    out = nc.dram_tensor("out", [TOK, H], F32, kind="ExternalOutput").ap()

    with tile.TileContext(nc) as tc, nc.allow_low_precision(
        reason="fp16 gate pipeline by design"
    ):
        _body(tc, nc, xT, wT, tT, out)

    nc.compile()
    return nc


def _body(tc, nc, xT, wT, tT, out):
    from contextlib import ExitStack

    from concourse.dve_ops import (
        RECIPROCAL_APPROX_FAST, RECIP_APPROX_FAST_CONSTS)

    ctx = ExitStack()
    with ctx:
        weights = ctx.enter_context(tc.tile_pool(name="weights", bufs=1))
        abuf_p = ctx.enter_context(tc.tile_pool(name="abuf", bufs=3))
        ebuf_p = ctx.enter_context(tc.tile_pool(name="ebuf", bufs=3))
        smalls = ctx.enter_context(tc.tile_pool(name="smalls", bufs=2))
        outs_p = ctx.enter_context(tc.tile_pool(name="outs", bufs=2))
        ppool = ctx.enter_context(tc.tile_pool(name="apsum", bufs=4, space="PSUM"))

        # ---- resident weights/activations (fp16, pre-transposed on host) ----
        xT_t, wT_t, tT_t = [], [], []
        for c in range(KC):
            xt = weights.tile([128, TOK], F16, name=f"xT{c}", tag=f"xT{c}")
            nc.sync.dma_start(out=xt[:], in_=xT[c * 128:(c + 1) * 128, :])
            xT_t.append(xt)
            wt = weights.tile([128, H], F16, name=f"wT{c}", tag=f"wT{c}")
            nc.sync.dma_start(out=wt[:], in_=wT[c * 128:(c + 1) * 128, :])
            wT_t.append(wt)
            tt_ = weights.tile([128, H], F16, name=f"tT{c}", tag=f"tT{c}")
            nc.sync.dma_start(out=tt_[:], in_=tT[c * 128:(c + 1) * 128, :])
            tT_t.append(tt_)

        for tt in range(NTT):
            tok_sl = bass.ts(tt, 128)
            for hc in range(NHC):
                h_sl = bass.ts(hc, 512)

                # ------- gate einsum + linear, interleaved on PE -------
                # (one lin k-chunk after each einsum pair keeps PE's pair
                # production smooth; a lin burst would starve the ACT
                # copy pipeline at unit start)
                lin_ps = ppool.tile([128, 2, 512], F32, tag="apair")
                halves = []
                for half in range(2):
                    a_bf = abuf_p.tile([128, HALF, 512], F16, tag="a_bf")
                    ebuf = ebuf_p.tile([128, HALF, 512], F16, tag="ebuf")
                    for pr in range(HALF // 2):
                        w0 = half * HALF + pr * 2
                        aps = ppool.tile([128, 2, 512], F32, tag="apair")
                        for i in range(2):
                            w = w0 + i
                            ct, ro = w // 2, (w % 2) * WIN
                            nc.tensor.matmul(
                                aps[:, i, :],
                                lhsT=xT_t[ct][ro:ro + WIN, tok_sl],
                                rhs=tT_t[ct][ro:ro + WIN, h_sl],
                                start=True, stop=True,
                            )
                        k = half * 8 + pr
                        nc.tensor.matmul(
                            lin_ps[:, 0, :],
                            lhsT=xT_t[k][:, tok_sl],
                            rhs=wT_t[k][:, h_sl],
                            start=(k == 0), stop=(k == KC - 1),
                            skip_group_check=True,
                        )
                        # fp32 PSUM -> fp16 SBUF on ACT (frees the psum
                        # pair; DVE is the bottleneck engine, keep it off)
                        nc.scalar.copy(
                            out=a_bf[:, pr * 2:pr * 2 + 2, :],
                            in_=aps[:, :, :],
                        )
                        # |a| -> ebuf per quarter: clear fp16 sign bit
                        # (DVE int16 4x mode, batched to amortize overhead)
                        if pr % 4 == 3:
                            qs = slice(pr * 2 - 6, pr * 2 + 2)
                            nc.vector.tensor_scalar(
                                out=ebuf[:, qs, :].bitcast(mybir.dt.uint16),
                                in0=a_bf[:, qs, :].bitcast(mybir.dt.uint16),
                                scalar1=0x7FFF, scalar2=None,
                                op0=mybir.AluOpType.bitwise_and,
                            )
                    # e = exp(|a|) and prod = a*e, quarter-granular so the
                    # DVE mul overlaps the ACT exp of the next quarter
                    for q in range(2):
                        qs = slice(q * 8, (q + 1) * 8)
                        nc.scalar.activation(
                            out=ebuf[:, qs, :], in_=ebuf[:, qs, :],
                            func=mybir.ActivationFunctionType.Exp,
                        )
                        nc.vector.tensor_tensor(
                            out=a_bf[:, qs, :], in0=a_bf[:, qs, :],
                            in1=ebuf[:, qs, :], op=mybir.AluOpType.mult,
                        )
                    # pairwise trees over windows: num in a_bf, den in ebuf
                    # (both DVE: GPSIMD shares an SBUF port with DVE and
                    # measured 3-4x slower - offloading there hurts).
                    # Last level lands in a small combined [num|den] tile
                    # so the big buffers free early.
                    n = HALF // 2
                    while n >= 2:
                        nc.vector.tensor_tensor(
                            out=a_bf[:, 0:n, :], in0=a_bf[:, 0:n, :],
                            in1=a_bf[:, n:2 * n, :], op=mybir.AluOpType.add,
                        )
                        nc.vector.tensor_tensor(
                            out=ebuf[:, 0:n, :], in0=ebuf[:, 0:n, :],
                            in1=ebuf[:, n:2 * n, :], op=mybir.AluOpType.add,
                        )
                        n //= 2
                    nd_h = smalls.tile([128, 2, 512], F16, tag=f"ndh{half}",
                                       name=f"ndh{half}")
                    nc.vector.tensor_tensor(
                        out=nd_h[:, 0, :], in0=a_bf[:, 0, :], in1=a_bf[:, 1, :],
                        op=mybir.AluOpType.add)
                    nc.vector.tensor_tensor(
                        out=nd_h[:, 1, :], in0=ebuf[:, 0, :], in1=ebuf[:, 1, :],
                        op=mybir.AluOpType.add)
                    halves.append(nd_h)

                # merge halves (single fused op), then g = num/den
                nd = halves[0]
                nc.vector.tensor_tensor(
                    out=nd[:], in0=nd[:], in1=halves[1][:],
                    op=mybir.AluOpType.add)
                rcp = smalls.tile([128, 512], F16, tag="rcp")
                nc.vector._custom_dve(
                    RECIPROCAL_APPROX_FAST, out=rcp[:], in0=nd[:, 1, :],
                    **RECIP_APPROX_FAST_CONSTS)
                g = smalls.tile([128, 512], F16, tag="g")
                nc.vector.tensor_tensor(
                    out=g[:], in0=nd[:, 0, :], in1=rcp[:],
                    op=mybir.AluOpType.mult)

                # lin PSUM -> fp16 SBUF (must precede the finish reads)
                lin = smalls.tile([128, 512], F16, tag="lin")
                nc.scalar.copy(out=lin[:], in_=lin_ps[:, 0, :])

                # ---- finish: out = g*lin*(1+tanh(g/2))/2 = silu(g)*lin ----
                # tanh shares the ACT table set with Exp: no table switches.
                # Recycle dead tiles (rcp, nd) to save SBUF.
                th = rcp
                nc.scalar.activation(
                    out=th[:], in_=g[:],
                    func=mybir.ActivationFunctionType.Tanh, scale=0.5,
                )
                gl = nd[:, 0, :]
                nc.vector.scalar_tensor_tensor(
                    out=gl, in0=g[:], scalar=0.5, in1=lin[:],
                    op0=mybir.AluOpType.mult, op1=mybir.AluOpType.mult)
                o = outs_p.tile([128, 512], F32, tag="o")
                nc.vector.scalar_tensor_tensor(
                    out=o[:], in0=th[:], scalar=1.0, in1=gl,
                    op0=mybir.AluOpType.add, op1=mybir.AluOpType.mult)
                nc.sync.dma_start(
                    out=out[bass.ts(tt, 128), bass.ts(hc, 512)], in_=o[:])


_NC_CACHE = None


def _get_module():
    global _NC_CACHE
    if _NC_CACHE is None:
        _NC_CACHE = _build_module()
    return _NC_CACHE


def kernel(x: np.ndarray, template_flat: np.ndarray,
           weights: np.ndarray) -> np.ndarray:
    nc = _get_module()

    xT = np.ascontiguousarray(x.T.astype(np.float16))           # [D, N]
    wT = np.ascontiguousarray(weights.T.astype(np.float16))     # [D, H]
    tT = np.ascontiguousarray(template_flat.T.astype(np.float16))

    in_maps = []
    for c in range(NCORES):
        in_maps.append({
            "xT": np.ascontiguousarray(xT[:, c * TOK:(c + 1) * TOK]),
            "wT": wT,
            "tT": tT,
        })
    res = run_bass_kernel_spmd(nc, in_maps, core_ids=list(range(NCORES)))
    return np.concatenate([res.results[c]["out"] for c in range(NCORES)],
                          axis=0).astype(np.float32)



# revision 5
# speedup vs baseline: 1.0338x; 1.0338x over previous
"""Trainium2 Bass kernel for nn_DendriticLayerSiLU_Template.

out = silu(g) * (x @ W.T), where per (token n, unit h):
  a[n,h,w] = sum_s x[n, w*64+s] * T[h, w*64+s]      (32 windows of size 64)
  p = softmax(|a| / tau), tau=1  (over w)
  g[n,h] = sum_w p[n,h,w] * a[n,h,w]

Strategy: 8-way data-parallel over N=4096 tokens (512/core), fp16 on-chip.
The gate's elementwise pipeline is the wall (ACT: PSUM drain + exp; DVE:
abs + a*e + reduction trees), so the kernel is software-pipelined at
half-unit (16-window) granularity with one slot of lookahead:

  slot k:  PE   einsum half k (16 windows, pair matmuls -> PSUM pairs)
           ACT  drain pairs of half k, exp(|a|) of half k
           DVE  mult+trees of half k-1 interleaved with abs of half k as
                drains land, plus the finished unit's tail every 2nd slot

lin = x@W.T runs as one PE burst per token-tile inside slots 1-4 (PE is
~30% busy) using a dedicated 2-bank PSUM tile; DMA streams inputs as
(xT_c, tT_c, wT_c) triplets so the first einsum starts within ~2us.
"""

import sys

if "/opt/trn_rl_repo" not in sys.path:
    sys.path.insert(0, "/opt/trn_rl_repo")

import numpy as np

import concourse.bass as bass
import concourse.tile as tile
from concourse import bacc, mybir
from concourse.bass_utils import run_bass_kernel_spmd

# Problem shapes (hardcoded per harness contract)
N_TOKENS = 4096
D = 2048          # in_features
H = 1024          # out_features
WIN = 64          # window size
NW = 32           # num windows
NCORES = 8
TOK = N_TOKENS // NCORES    # tokens per core = 512
NTT = TOK // 128            # token tiles per core = 4
NHC = H // 512              # h chunks = 2
KC = D // 128               # k chunks for linear = 16

F16 = mybir.dt.float16
F32 = mybir.dt.float32
U16 = mybir.dt.uint16


def _build_module():
    nc = bacc.Bacc("TRN2", target_bir_lowering=False, debug=False,
                   num_devices=NCORES)

    xT = nc.dram_tensor("xT", [D, TOK], F16, kind="ExternalInput").ap()
    wT = nc.dram_tensor("wT", [D, H], F16, kind="ExternalInput").ap()
    tT = nc.dram_tensor("tT", [D, H], F16, kind="ExternalInput").ap()
    out = nc.dram_tensor("out", [TOK, H], F32, kind="ExternalOutput").ap()

    with tile.TileContext(nc) as tc, nc.allow_low_precision(
        reason="fp16 gate pipeline by design"
    ):
        _body(tc, nc, xT, wT, tT, out)

    nc.compile()
    return nc


class _HalfSlot:
    """One 16-window half-unit: the nd tile plus unit bookkeeping."""

    def __init__(self, nd, ndh, half, tt, hc):
        self.nd = nd       # [128, 2(a|e), 16, 512] fp16
        self.ndh = ndh     # per-unit [128, 2(half), 2(num|den), 512]
        self.half = half
        self.tt = tt
        self.hc = hc


def _body(tc, nc, xT, wT, tT, out):
    from contextlib import ExitStack

    from concourse.dve_ops import (
        RECIPROCAL_APPROX_FAST, RECIP_APPROX_FAST_CONSTS)

    ctx = ExitStack()
    with ctx:
        weights = ctx.enter_context(tc.tile_pool(name="weights", bufs=1))
        nd_p = ctx.enter_context(tc.tile_pool(name="nd", bufs=2))
        smalls = ctx.enter_context(tc.tile_pool(name="smalls", bufs=3))
        outs_p = ctx.enter_context(tc.tile_pool(name="outs", bufs=2))
        ppool = ctx.enter_context(tc.tile_pool(name="apsum", bufs=3,
                                               space="PSUM"))
        lpool = ctx.enter_context(tc.tile_pool(name="lpsum", bufs=1,
                                               space="PSUM"))

        # ---- resident inputs (fp16, pre-transposed on host) ----
        # DMA in (xT_c, tT_c, wT_c) triplets: the slot-0 einsum needs only
        # xT/tT chunks 0-7, so PE starts within ~2us; wT arrives in time
        # for the lin bursts in slots 1-4.
        xT_t, wT_t, tT_t = [], [], []
        for c in range(KC):
            xt = weights.tile([128, TOK], F16, name=f"xT{c}", tag=f"xT{c}")
            nc.sync.dma_start(out=xt[:], in_=xT[c * 128:(c + 1) * 128, :])
            xT_t.append(xt)
            tt_ = weights.tile([128, H], F16, name=f"tT{c}", tag=f"tT{c}")
            nc.sync.dma_start(out=tt_[:], in_=tT[c * 128:(c + 1) * 128, :])
            tT_t.append(tt_)
            wt = weights.tile([128, H], F16, name=f"wT{c}", tag=f"wT{c}")
            nc.sync.dma_start(out=wt[:], in_=wT[c * 128:(c + 1) * 128, :])
            wT_t.append(wt)

        lin_bf = [weights.tile([128, 2, 512], F16, name=f"lin{t}",
                               tag=f"lin{t}") for t in range(NTT)]

        # ---- helpers -----------------------------------------------------
        def emit_pe_drains(slot):
            """PE einsum pairs + ACT pair drains for half-slot `slot`."""
            nd, tt, hc, half = slot.nd, slot.tt, slot.hc, slot.half
            tok_sl = bass.ts(tt, 128)
            h_sl = bass.ts(hc, 512)
            for pr in range(8):
                aps = ppool.tile([128, 2, 512], F32, tag="aps", name="aps")
                for i in range(2):
                    w = half * 16 + pr * 2 + i
                    ct, ro = w // 2, (w % 2) * WIN
                    nc.tensor.matmul(
                        aps[:, i, :],
                        lhsT=xT_t[ct][ro:ro + WIN, tok_sl],
                        rhs=tT_t[ct][ro:ro + WIN, h_sl],
                        start=True, stop=True,
                    )
                nc.scalar.copy(out=nd[:, 0, pr * 2:pr * 2 + 2, :],
                               in_=aps[:, :, :])

        def emit_abs(slot, grp):
            """|a| -> plane 1 for one 4-window group (DVE int16 4x mode)."""
            gs = slice(grp * 4, grp * 4 + 4)
            nd = slot.nd
            nc.vector.tensor_scalar(
                out=nd[:, 1, gs, :].bitcast(U16),
                in0=nd[:, 0, gs, :].bitcast(U16),
                scalar1=0x7FFF, scalar2=None,
                op0=mybir.AluOpType.bitwise_and,
            )

        def emit_exp(slot, q):
            """e = exp(|a|) in place on plane 1 (8-window ACT op)."""
            qs = slice(q * 8, (q + 1) * 8)
            nd = slot.nd
            nc.scalar.activation(
                out=nd[:, 1, qs, :], in_=nd[:, 1, qs, :],
                func=mybir.ActivationFunctionType.Exp,
            )

        def emit_mult(slot, q):
            """prod = a * e in-place on plane 0 (8-window tensor_tensor)."""
            qs = slice(q * 8, (q + 1) * 8)
            nd = slot.nd
            nc.vector.tensor_tensor(
                out=nd[:, 0, qs, :], in0=nd[:, 0, qs, :],
                in1=nd[:, 1, qs, :], op=mybir.AluOpType.mult,
            )

        def emit_tree(slot):
            """Per-plane pairwise tree; result -> ndh[:, half]."""
            nd = slot.nd
            n = 8
            while n >= 2:
                for p in range(2):
                    nc.vector.tensor_tensor(
                        out=nd[:, p, 0:n, :], in0=nd[:, p, 0:n, :],
                        in1=nd[:, p, n:2 * n, :], op=mybir.AluOpType.add,
                    )
                n //= 2
            for p in range(2):
                nc.vector.tensor_tensor(
                    out=slot.ndh[:, slot.half, p, :],
                    in0=nd[:, p, 0, :], in1=nd[:, p, 1, :],
                    op=mybir.AluOpType.add,
                )

        def emit_lin(t):
            """One token tile of lin = x @ W.T: PE burst + ACT drain."""
            tok_sl = bass.ts(t, 128)
            lps = lpool.tile([128, 2, 512], F32, tag="lps", name="lps")
            for hc in range(NHC):
                for k in range(KC):
                    nc.tensor.matmul(
                        lps[:, hc, :],
                        lhsT=xT_t[k][:, tok_sl],
                        rhs=wT_t[k][:, bass.ts(hc, 512)],
                        start=(k == 0), stop=(k == KC - 1),
                    )
            nc.scalar.copy(out=lin_bf[t][:], in_=lps[:, :, :])

        def emit_tail(slot):
            """Merge halves, g = num/den, out = silu(g)*lin, DMA."""
            ndh, tt, hc = slot.ndh, slot.tt, slot.hc
            for p in range(2):
                nc.vector.tensor_tensor(
                    out=ndh[:, 0, p, :], in0=ndh[:, 0, p, :],
                    in1=ndh[:, 1, p, :], op=mybir.AluOpType.add)
            rcp = smalls.tile([128, 512], F16, tag="rcp")
            nc.vector._custom_dve(
                RECIPROCAL_APPROX_FAST, out=rcp[:], in0=ndh[:, 0, 1, :],
                **RECIP_APPROX_FAST_CONSTS)
            g = smalls.tile([128, 512], F16, tag="g")
            nc.vector.tensor_tensor(
                out=g[:], in0=ndh[:, 0, 0, :], in1=rcp[:],
                op=mybir.AluOpType.mult)
            # silu(g) = g * (1 + tanh(g/2)) / 2; tanh shares the exp ACT
            # table set -> no table switches.
            th = rcp  # recycle
            nc.scalar.activation(
                out=th[:], in_=g[:],
                func=mybir.ActivationFunctionType.Tanh, scale=0.5,
            )
            gl = ndh[:, 0, 0, :]  # recycle dead slot
            nc.vector.scalar_tensor_tensor(
                out=gl, in0=g[:], scalar=0.5, in1=lin_bf[tt][:, hc, :],
                op0=mybir.AluOpType.mult, op1=mybir.AluOpType.mult)
            o = outs_p.tile([128, 512], F32, tag="o")
            nc.vector.scalar_tensor_tensor(
                out=o[:], in0=th[:], scalar=1.0, in1=gl,
                op0=mybir.AluOpType.add, op1=mybir.AluOpType.mult)
            nc.sync.dma_start(
                out=out[bass.ts(tt, 128), bass.ts(hc, 512)], in_=o[:])

        # ---- main loop: 16 half-slots, one slot of lookahead --------------
        prev = None        # _HalfSlot whose DVE mult/tree runs this slot
        tail_ready = None  # unit whose tail runs this slot
        ndh_cur = None
        slot_idx = 0
        for tt in range(NTT):
            for hc in range(NHC):
                for half in range(2):
                    if half == 0:
                        ndh_cur = smalls.tile([128, 2, 2, 512], F16,
                                              tag="ndh")
                    nd_tile = nd_p.tile([128, 2, 16, 512], F16, tag="nd",
                                        name="nd")
                    cur = _HalfSlot(nd_tile, ndh_cur, half, tt, hc)
                    emit_pe_drains(cur)
                    # DVE: prev's mults interleaved with cur's abs groups
                    if prev is not None:
                        emit_mult(prev, 0)
                        emit_abs(cur, 0)
                        emit_abs(cur, 1)
                        emit_mult(prev, 1)
                        emit_abs(cur, 2)
                        emit_abs(cur, 3)
                    else:
                        for g_ in range(4):
                            emit_abs(cur, g_)
                    # ACT: exps for cur (after cur's abs in program order)
                    emit_exp(cur, 0)
                    emit_exp(cur, 1)
                    # lin bursts occupy PE/ACT slack in slots 1-4
                    if 1 <= slot_idx <= NTT:
                        emit_lin(slot_idx - 1)
                    # DVE: prev's tree, then the finished unit's tail
                    if prev is not None:
                        emit_tree(prev)
                    finished = prev if (prev is not None
                                        and prev.half == 1) else None
                    if tail_ready is not None:
                        emit_tail(tail_ready)
                    tail_ready = finished
                    prev = cur
                    slot_idx += 1

        # ---- pipeline flush ----
        emit_mult(prev, 0)
        emit_mult(prev, 1)
        emit_tree(prev)
        if tail_ready is not None:
            emit_tail(tail_ready)
        emit_tail(prev)


_NC_CACHE = None


def _get_module():
    global _NC_CACHE
    if _NC_CACHE is None:
        _NC_CACHE = _build_module()
    return _NC_CACHE


def kernel(x: np.ndarray, template_flat: np.ndarray,
           weights: np.ndarray) -> np.ndarray:
    nc = _get_module()

    xT = np.ascontiguousarray(x.T.astype(np.float16))           # [D, N]
    wT = np.ascontiguousarray(weights.T.astype(np.float16))     # [D, H]
    tT = np.ascontiguousarray(template_flat.T.astype(np.float16))

    in_maps = []
    for c in range(NCORES):
        in_maps.append({
            "xT": np.ascontiguousarray(xT[:, c * TOK:(c + 1) * TOK]),
            "wT": wT,
            "tT": tT,
        })
    res = run_bass_kernel_spmd(nc, in_maps, core_ids=list(range(NCORES)))
    return np.concatenate([res.results[c]["out"] for c in range(NCORES)],
                          axis=0).astype(np.float32)


# revision 9
# speedup vs baseline: 1.0396x; 1.0056x over previous
"""Trainium2 Bass kernel for nn_DendriticLayerSiLU_Template.

out = silu(g) * (x @ W.T), where per (token n, unit h):
  a[n,h,w] = sum_s x[n, w*64+s] * T[h, w*64+s]      (32 windows of size 64)
  p = softmax(|a| / tau), tau=1  (over w)
  g[n,h] = sum_w p[n,h,w] * a[n,h,w]

Strategy: 8-way data-parallel over N=4096 tokens (512/core), fp16 on-chip.
The gate's elementwise pipeline is the wall (ACT: PSUM drain + exp; DVE:
abs + a*e + reduction trees), so the kernel is software-pipelined at
half-unit (16-window) granularity with one slot of lookahead:

  slot k:  PE   einsum half k (16 windows, pair matmuls -> PSUM pairs)
           ACT  drain pairs of half k, exp(|a|) of half k
           DVE  mult+trees of half k-1 interleaved with abs of half k as
                drains land, plus the finished unit's tail every 2nd slot

lin = x@W.T runs as one PE burst per token-tile inside slots 1-4 (PE is
~30% busy) using a dedicated 2-bank PSUM tile; DMA streams inputs as
(xT_c, tT_c, wT_c) triplets so the first einsum starts within ~2us.
"""

import sys

if "/opt/trn_rl_repo" not in sys.path:
    sys.path.insert(0, "/opt/trn_rl_repo")

import numpy as np

import concourse.bass as bass
import concourse.tile as tile
from concourse import bacc, mybir
from concourse.bass_utils import run_bass_kernel_spmd

# Problem shapes (hardcoded per harness contract)
N_TOKENS = 4096
D = 2048          # in_features
H = 1024          # out_features
WIN = 64          # window size
NW = 32           # num windows
NCORES = 8
TOK = N_TOKENS // NCORES    # tokens per core = 512
NTT = TOK // 128            # token tiles per core = 4
NHC = H // 512              # h chunks = 2
KC = D // 128               # k chunks for linear = 16

F16 = mybir.dt.float16
F32 = mybir.dt.float32
U16 = mybir.dt.uint16


def _build_module():
    nc = bacc.Bacc("TRN2", target_bir_lowering=False, debug=False,
                   num_devices=NCORES)

    xT = nc.dram_tensor("xT", [D, TOK], F16, kind="ExternalInput").ap()
    wT = nc.dram_tensor("wT", [D, H], F16, kind="ExternalInput").ap()
    tT = nc.dram_tensor("tT", [D, H], F16, kind="ExternalInput").ap()
    out = nc.dram_tensor("out", [TOK, H], F32, kind="ExternalOutput").ap()

    with tile.TileContext(nc) as tc, nc.allow_low_precision(
        reason="fp16 gate pipeline by design"
    ):
        _body(tc, nc, xT, wT, tT, out)

    nc.compile()
    return nc


class _HalfSlot:
    """One 16-window half-unit: the nd tile plus unit bookkeeping."""

    def __init__(self, nd, ndh, half, tt, hc):
        self.nd = nd       # [128, 2(a|e), 16, 512] fp16
        self.ndh = ndh     # per-unit [128, 2(half), 2(num|den), 512]
        self.half = half
        self.tt = tt
        self.hc = hc


def _body(tc, nc, xT, wT, tT, out):
    from contextlib import ExitStack

    from concourse.dve_ops import (
        RECIPROCAL_APPROX_FAST, RECIP_APPROX_FAST_CONSTS)

    ctx = ExitStack()
    with ctx:
        weights = ctx.enter_context(tc.tile_pool(name="weights", bufs=1))
        nd_p = ctx.enter_context(tc.tile_pool(name="nd", bufs=2))
        smalls = ctx.enter_context(tc.tile_pool(name="smalls", bufs=3))
        outs_p = ctx.enter_context(tc.tile_pool(name="outs", bufs=2))
        ppool = ctx.enter_context(tc.tile_pool(name="apsum", bufs=3,
                                               space="PSUM"))
        lpool = ctx.enter_context(tc.tile_pool(name="lpsum", bufs=1,
                                               space="PSUM"))

        # ---- resident inputs (fp16, pre-transposed on host) ----
        # DMA in (xT_c, tT_c, wT_c) triplets: the slot-0 einsum needs only
        # xT/tT chunks 0-7, so PE starts within ~2us; wT arrives in time
        # for the lin bursts in slots 1-4.
        xT_t, wT_t, tT_t = [], [], []
        for c in range(KC):
            xt = weights.tile([128, TOK], F16, name=f"xT{c}", tag=f"xT{c}")
            nc.sync.dma_start(out=xt[:], in_=xT[c * 128:(c + 1) * 128, :])
            xT_t.append(xt)
            tt_ = weights.tile([128, H], F16, name=f"tT{c}", tag=f"tT{c}")
            nc.sync.dma_start(out=tt_[:], in_=tT[c * 128:(c + 1) * 128, :])
            tT_t.append(tt_)
            wt = weights.tile([128, H], F16, name=f"wT{c}", tag=f"wT{c}")
            nc.sync.dma_start(out=wt[:], in_=wT[c * 128:(c + 1) * 128, :])
            wT_t.append(wt)

        lin_bf = [weights.tile([128, 2, 512], F16, name=f"lin{t}",
                               tag=f"lin{t}") for t in range(NTT)]

        # ---- helpers -----------------------------------------------------
        def emit_pe_drains(slot):
            """PE einsum pairs + ACT pair drains for half-slot `slot`."""
            nd, tt, hc, half = slot.nd, slot.tt, slot.hc, slot.half
            tok_sl = bass.ts(tt, 128)
            h_sl = bass.ts(hc, 512)
            for pr in range(8):
                aps = ppool.tile([128, 2, 512], F32, tag="aps", name="aps")
                for i in range(2):
                    w = half * 16 + pr * 2 + i
                    ct, ro = w // 2, (w % 2) * WIN
                    nc.tensor.matmul(
                        aps[:, i, :],
                        lhsT=xT_t[ct][ro:ro + WIN, tok_sl],
                        rhs=tT_t[ct][ro:ro + WIN, h_sl],
                        start=True, stop=True,
                    )
                nc.scalar.copy(out=nd[:, 0, pr * 2:pr * 2 + 2, :],
                               in_=aps[:, :, :])

        def emit_abs(slot, grp):
            """|a| -> plane 1 for one 4-window group (DVE int16 4x mode)."""
            gs = slice(grp * 4, grp * 4 + 4)
            nd = slot.nd
            nc.vector.tensor_scalar(
                out=nd[:, 1, gs, :].bitcast(U16),
                in0=nd[:, 0, gs, :].bitcast(U16),
                scalar1=0x7FFF, scalar2=None,
                op0=mybir.AluOpType.bitwise_and,
            )

        def emit_exp(slot, q):
            """e = exp(|a|) in place on plane 1 (8-window ACT op)."""
            qs = slice(q * 8, (q + 1) * 8)
            nd = slot.nd
            nc.scalar.activation(
                out=nd[:, 1, qs, :], in_=nd[:, 1, qs, :],
                func=mybir.ActivationFunctionType.Exp,
            )

        def emit_mult(slot, q):
            """prod = a * e in-place on plane 0 (8-window tensor_tensor)."""
            qs = slice(q * 8, (q + 1) * 8)
            nd = slot.nd
            nc.vector.tensor_tensor(
                out=nd[:, 0, qs, :], in0=nd[:, 0, qs, :],
                in1=nd[:, 1, qs, :], op=mybir.AluOpType.mult,
            )

        def emit_tree(slot):
            """Per-plane pairwise tree; result -> ndh[:, half]."""
            nd = slot.nd
            n = 8
            while n >= 2:
                for p in range(2):
                    nc.vector.tensor_tensor(
                        out=nd[:, p, 0:n, :], in0=nd[:, p, 0:n, :],
                        in1=nd[:, p, n:2 * n, :], op=mybir.AluOpType.add,
                    )
                n //= 2
            for p in range(2):
                nc.vector.tensor_tensor(
                    out=slot.ndh[:, slot.half, p, :],
                    in0=nd[:, p, 0, :], in1=nd[:, p, 1, :],
                    op=mybir.AluOpType.add,
                )

        def emit_lin(t):
            """One token tile of lin = x @ W.T: PE burst + ACT drain."""
            tok_sl = bass.ts(t, 128)
            lps = lpool.tile([128, 2, 512], F32, tag="lps", name="lps")
            for hc in range(NHC):
                for k in range(KC):
                    nc.tensor.matmul(
                        lps[:, hc, :],
                        lhsT=xT_t[k][:, tok_sl],
                        rhs=wT_t[k][:, bass.ts(hc, 512)],
                        start=(k == 0), stop=(k == KC - 1),
                    )
            nc.scalar.copy(out=lin_bf[t][:], in_=lps[:, :, :])

        def emit_tail(slot):
            """Merge halves, g = num/den, out = silu(g)*lin, DMA."""
            ndh, tt, hc = slot.ndh, slot.tt, slot.hc
            for p in range(2):
                nc.vector.tensor_tensor(
                    out=ndh[:, 0, p, :], in0=ndh[:, 0, p, :],
                    in1=ndh[:, 1, p, :], op=mybir.AluOpType.add)
            rcp = smalls.tile([128, 512], F16, tag="rcp")
            nc.vector._custom_dve(
                RECIPROCAL_APPROX_FAST, out=rcp[:], in0=ndh[:, 0, 1, :],
                **RECIP_APPROX_FAST_CONSTS)
            g = smalls.tile([128, 512], F16, tag="g")
            nc.vector.tensor_tensor(
                out=g[:], in0=ndh[:, 0, 0, :], in1=rcp[:],
                op=mybir.AluOpType.mult)
            # silu(g) = g * (1 + tanh(g/2)) / 2; tanh shares the exp ACT
            # table set -> no table switches.
            th = rcp  # recycle
            nc.scalar.activation(
                out=th[:], in_=g[:],
                func=mybir.ActivationFunctionType.Tanh, scale=0.5,
            )
            gl = ndh[:, 0, 0, :]  # recycle dead slot
            nc.vector.scalar_tensor_tensor(
                out=gl, in0=g[:], scalar=0.5, in1=lin_bf[tt][:, hc, :],
                op0=mybir.AluOpType.mult, op1=mybir.AluOpType.mult)
            o = outs_p.tile([128, 512], F32, tag="o")
            nc.vector.scalar_tensor_tensor(
                out=o[:], in0=th[:], scalar=1.0, in1=gl,
                op0=mybir.AluOpType.add, op1=mybir.AluOpType.mult)
            nc.sync.dma_start(
                out=out[bass.ts(tt, 128), bass.ts(hc, 512)], in_=o[:])

        # ---- main loop: 16 half-slots, one slot of lookahead --------------
        prev = None        # _HalfSlot whose DVE mult/tree runs this slot
        ndh_cur = None
        slot_idx = 0
        for tt in range(NTT):
            for hc in range(NHC):
                for half in range(2):
                    if half == 0:
                        ndh_cur = smalls.tile([128, 2, 2, 512], F16,
                                              tag="ndh")
                    nd_tile = nd_p.tile([128, 2, 16, 512], F16, tag="nd",
                                        name="nd")
                    cur = _HalfSlot(nd_tile, ndh_cur, half, tt, hc)
                    emit_pe_drains(cur)
                    # DVE: prev's mults interleaved with cur's abs groups
                    if prev is not None:
                        emit_mult(prev, 0)
                        emit_abs(cur, 0)
                        emit_abs(cur, 1)
                        emit_mult(prev, 1)
                        emit_abs(cur, 2)
                        emit_abs(cur, 3)
                    else:
                        for g_ in range(4):
                            emit_abs(cur, g_)
                    # ACT: exps for cur (after cur's abs in program order)
                    emit_exp(cur, 0)
                    emit_exp(cur, 1)
                    # lin bursts occupy PE/ACT slack in slots 2-5: late
                    # enough that wT has streamed in, and slot 2 emits
                    # lin(tt0) just before unit (tt0,hc0)'s tail uses it
                    if 2 <= slot_idx <= NTT + 1:
                        emit_lin(slot_idx - 2)
                    # DVE: prev's tree, then the finished unit's tail
                    if prev is not None:
                        emit_tree(prev)
                        if prev.half == 1:
                            emit_tail(prev)
                    prev = cur
                    slot_idx += 1

        # ---- pipeline flush ----
        emit_mult(prev, 0)
        emit_mult(prev, 1)
        emit_tree(prev)
        emit_tail(prev)


_NC_CACHE = None


def _get_module():
    global _NC_CACHE
    if _NC_CACHE is None:
        _NC_CACHE = _build_module()
    return _NC_CACHE


def kernel(x: np.ndarray, template_flat: np.ndarray,
           weights: np.ndarray) -> np.ndarray:
    nc = _get_module()

    xT = np.ascontiguousarray(x.T.astype(np.float16))           # [D, N]
    wT = np.ascontiguousarray(weights.T.astype(np.float16))     # [D, H]
    tT = np.ascontiguousarray(template_flat.T.astype(np.float16))

    in_maps = []
    for c in range(NCORES):
        in_maps.append({
            "xT": np.ascontiguousarray(xT[:, c * TOK:(c + 1) * TOK]),
            "wT": wT,
            "tT": tT,
        })
    res = run_bass_kernel_spmd(nc, in_maps, core_ids=list(range(NCORES)))
    return np.concatenate([res.results[c]["out"] for c in range(NCORES)],
                          axis=0).astype(np.float32)
